# revision 31
# baseline (speedup 1.0000x reference)
"""Multi-head self-attention (RoPE, causal) Trainium2 Bass kernel — v7.

Sharding: head-parallel across 8 NeuronCores. Core c owns heads {2c, 2c+1}
for both batch rows. Each core computes its heads' QKV projection, RoPE,
causal flash attention (scores kept transposed [k, q]), softmax
normalization, and a partial output projection against its 128 columns of
W_o. The host sums the 8 partial projections (the "all-reduce").

v7 structure (engines execute in emission order — emission is the schedule):
- token-major phasing: project only batch-0 token chunks (Q1,K1,V1), rope
  b0, then start b0 scores ~30us earlier; second-half projections Q2/K2/V2
  and rope-b1 are enqueued as fillers consumed between b0 score pairs
- per-j PSUM-accumulated AV chains, h1 packed at out-partitions 64-127,
  exp-sums via ones columns at partitions 64 (h0) / 32 (h1)
- score matmul pairs row-tiled (h0 rows 0-63, h1 rows 64-127)
- ACT queue carries only exp (+ rare sums copies) so the 352-cycle
  activation setup pipelines away
- PSUM handoff: scp(4)+qps(4) during projections, qps released after b0's
  first score group, then avp(2)+yps(2)
- fully contiguous per-partition DMA chunks; output yt [P, jj, et, 512]

Self-contained: hardcodes B=2, S=2048, D=1024, H=16, d_k=64.
"""
import numpy as np
import ml_dtypes
from collections import deque

B, S, D, H, DK = 2, 2048, 1024, 16, 64
NCORES = 8
HPC = H // NCORES            # 2 heads per core
THETA = 10000.0
BS = B * S                   # 4096 flattened tokens (b-major)
KT = D // 128                # 8 contraction tiles
P = 128

bf16 = ml_dtypes.bfloat16

_CACHED_NC = None


def _host_prep(x, token_positions, W_qkv, W_o):
    """Build per-core DRAM input dicts (numpy, bf16), partition-major."""
    cast = lambda a: np.ascontiguousarray(a).astype(bf16)
    X2 = np.asarray(x, np.float32).reshape(BS, D)
    # [P, chunk, KT, 512]: per-partition-contiguous 8KB chunks
    xt = cast(X2.T.reshape(KT, P, 8, 512).transpose(1, 2, 0, 3))

    pos = np.asarray(token_positions, np.float64)
    inv = THETA ** (-np.arange(0, DK, 2, dtype=np.float64) / DK)   # [32]
    ang = pos[:, None] * inv[None, :]                              # [S, 32]
    cosv = np.cos(ang).T.astype(np.float32)                        # [32, S]
    sinv = np.sin(ang).T.astype(np.float32)
    COS = cast(np.tile(cosv, (4, 1)))                              # [128, S]
    SINS = cast(np.concatenate([-sinv, sinv, -sinv, sinv], 0))     # [128, S]

    perm = np.concatenate([np.arange(0, 64, 2), np.arange(1, 64, 2)])
    tri = cast(np.triu(np.ones((P, P), np.float32)))               # [k,q]: q>=k

    # recip broadcast selector: rows = SUMS4 rows, cols = d partitions.
    # SUMS4 rows: 0=(h1, j_even) 1=(h1, j_odd) 2=(h0, j_even) 3=(h0, j_odd)
    sel8 = np.zeros((4, 2, 128), np.float32)
    sel8[2, 0, 0:64] = 1.0
    sel8[0, 0, 64:128] = 1.0
    sel8[3, 1, 0:64] = 1.0
    sel8[1, 1, 64:128] = 1.0

    def wslice(rows):
        # [P, KT, 128] partition-major
        w = np.asarray(W_qkv, np.float32)[rows].T.reshape(KT, P, P)
        return cast(w.transpose(1, 0, 2))

    Wo = np.asarray(W_o, np.float32)
    maps = []
    for c in range(NCORES):
        hA = HPC * c
        rows = np.concatenate([(hA + 0) * 64 + perm, (hA + 1) * 64 + perm])
        rows_v = np.concatenate([(hA + 0) * 64 + np.arange(64),
                                 (hA + 1) * 64 + np.arange(64)])
        maps.append({
            "xt": xt,
            "wq": wslice(rows),
            "wk": wslice(D + rows),
            "wv": wslice(2 * D + rows_v),
            "cos": COS,
            "sin": SINS,
            "tri": tri,
            "sel8": sel8,
            "wo": cast(Wo[:, P * c:P * c + P].T),                  # [128, 1024]
        })
    return maps


def _build_nc():
    """Trace + compile the per-core Bass module (same program on all cores)."""
    from contextlib import ExitStack
    import concourse.bacc as bacc
    import concourse.mybir as mybir
    import concourse.tile as tile
    from concourse.bass import ts

    f32 = mybir.dt.float32
    bf = mybir.dt.bfloat16
    EXP = mybir.ActivationFunctionType.Exp

    nc = bacc.Bacc("TRN2", target_bir_lowering=False, debug=False,
                   enable_asserts=False)

    xt_d = nc.dram_tensor("xt", [P, 8, KT, 512], bf, kind="ExternalInput").ap()
    wq_d = nc.dram_tensor("wq", [P, KT, P], bf, kind="ExternalInput").ap()
    wk_d = nc.dram_tensor("wk", [P, KT, P], bf, kind="ExternalInput").ap()
    wv_d = nc.dram_tensor("wv", [P, KT, P], bf, kind="ExternalInput").ap()
    wo_d = nc.dram_tensor("wo", [P, D], bf, kind="ExternalInput").ap()
    cos_d = nc.dram_tensor("cos", [P, S], bf, kind="ExternalInput").ap()
    sin_d = nc.dram_tensor("sin", [P, S], bf, kind="ExternalInput").ap()
    tri_d = nc.dram_tensor("tri", [P, P], bf, kind="ExternalInput").ap()
    sel_d = nc.dram_tensor("sel8", [4, 2, P], f32, kind="ExternalInput").ap()
    yt_d = nc.dram_tensor("yt", [P, 8, 8, 512], bf, kind="ExternalOutput").ap()

    with tile.TileContext(nc) as tc, ExitStack() as ctx:
        pp = ctx.enter_context(tc.tile_pool(name="persist", bufs=1))
        WO = pp.tile([P, D], bf, tag="wo")
        TRI = pp.tile([P, P], bf, tag="tri")
        SEL8 = pp.tile([4, 2, P], f32, tag="sel8")
        SROW = pp.tile([65, 4 * 512], f32, tag="srow")   # exp-sums rows
        SUMS4 = [pp.tile([4, 512], f32, tag=f"sums{g}", name=f"sums{g}")
                 for g in range(2)]
        RECIP4 = [pp.tile([4, 512], f32, tag=f"recip{g}", name=f"recip{g}")
                  for g in range(2)]
        ab = ctx.enter_context(tc.tile_pool(name="attnbuf", bufs=1))
        QA = ab.tile([P, BS], bf, tag="qa")
        KA = ab.tile([P, BS], bf, tag="ka")
        VT = ab.tile([P, BS], bf, tag="vt")

        for g in range(2):
            nc.gpsimd.memset(SUMS4[g][:], 1.0)
        nc.gpsimd.dma_start(WO[:], wo_d)
        nc.gpsimd.dma_start(TRI[:], tri_d)
        nc.gpsimd.dma_start(SEL8[:], sel_d)

        # ---- pools: weights/x/rope (xpA on the right stack, released
        # early so pt tiles reuse its space) ----
        xw = tc.alloc_tile_pool(name="xw", bufs=1)
        xpB = tc.alloc_tile_pool(name="xpB", bufs=1)
        swc = tc.alloc_tile_pool(name="swc", bufs=1)
        sws = tc.alloc_tile_pool(name="sws", bufs=2)
        xpA = tc.alloc_tile_pool(name="xpA", bufs=1, side="right")
        scp = tc.alloc_tile_pool(name="scps", bufs=4, space="PSUM")
        qps = tc.alloc_tile_pool(name="qkvps", bufs=4, space="PSUM")

        WQ = xw.tile([P, KT, P], bf, tag="wq")
        WK = xw.tile([P, KT, P], bf, tag="wk")
        WV = xw.tile([P, KT, P], bf, tag="wv")
        XTa = xpA.tile([P, 4, KT, 512], bf, tag="xta")
        XTb = xpB.tile([P, 4, KT, 512], bf, tag="xtb")
        COS = swc.tile([P, S], bf, tag="cos")
        SIN = swc.tile([P, S], bf, tag="sin")

        # input stream split across the two HWDGE queues, b0 chunks first
        nc.sync.dma_start(WQ[:], wq_d)
        nc.sync.dma_start(XTa[:, 0], xt_d[:, 0])
        nc.scalar.dma_start(XTa[:, 1], xt_d[:, 1])
        nc.scalar.dma_start(WK[:], wk_d)
        nc.sync.dma_start(XTa[:, 2], xt_d[:, 2])
        nc.scalar.dma_start(XTa[:, 3], xt_d[:, 3])
        nc.sync.dma_start(COS[:], cos_d)
        nc.scalar.dma_start(SIN[:], sin_d)
        nc.sync.dma_start(WV[:], wv_d)
        nc.sync.dma_start(XTb[:, 0], xt_d[:, 4])
        nc.scalar.dma_start(XTb[:, 1], xt_d[:, 5])
        nc.sync.dma_start(XTb[:, 2], xt_d[:, 6])
        nc.scalar.dma_start(XTb[:, 3], xt_d[:, 7])

        def proj_chunk(Wt, DST, j, drain="dve"):
            XT = XTa if j < 4 else XTb
            jc = j % 4
            ps = qps.tile([P, 512], f32, tag="qkv", name="ps")
            for kt in range(KT):
                nc.tensor.matmul(ps[:], lhsT=Wt[:, kt, :],
                                 rhs=XT[:, jc, kt, :],
                                 start=(kt == 0), stop=(kt == KT - 1))
            if drain == "act":
                nc.scalar.copy(DST[:, ts(j, 512)], ps[:])
            else:
                nc.vector.tensor_copy(DST[:, ts(j, 512)], ps[:])

        def rope_chunk(ch):
            ssl = ts(ch, 1024)
            csl = ts(ch % 2, 1024)
            QSc = sws.tile([P, 1024], bf, tag="qs", name="qsc")
            KSc = sws.tile([P, 1024], bf, tag="ks", name="ksc")
            for A, SWT in ((QA, QSc), (KA, KSc)):
                for blk in range(4):  # partition-block swap 0<->1, 2<->3
                    src = blk ^ 1
                    eng = nc.sync if blk % 2 == 0 else nc.scalar
                    eng.dma_start(SWT[32 * blk:32 * blk + 32, :],
                                  A[32 * src:32 * src + 32, ssl])
            for A, SWT in ((QA, QSc), (KA, KSc)):
                nc.vector.tensor_mul(A[:, ssl], A[:, ssl], COS[:, csl])
                nc.vector.tensor_mul(SWT[:], SWT[:], SIN[:, csl])
                nc.vector.tensor_add(A[:, ssl], A[:, ssl], SWT[:])

        # phase 1a: batch-0 halves only, then rope b0
        for j in range(4):
            proj_chunk(WQ, QA, j, drain="act")
        for j in range(4):
            proj_chunk(WK, KA, j, drain="act")
        rope_chunk(0)
        rope_chunk(1)
        for j in range(4):
            proj_chunk(WV, VT, j, drain="act")
        xpA.release()

        # ---- Phase 2: causal attention + o_proj, per-j chains ----
        ptw = [S - 512 * (i // 4) for i in range(16)]  # pt widths
        with tc.tile_pool(name="vb", bufs=2) as vbp, \
             tc.tile_pool(name="pt", bufs=1) as ptp, \
             tc.tile_pool(name="pasj", bufs=2) as pjp, \
             tc.tile_pool(name="opr", bufs=2) as orp, \
             tc.tile_pool(name="yb", bufs=2) as ybp:

            avp = None
            yps = None
            fq = deque()           # entries: (kind, thunk)

            def pop_fill(n=1):
                for _ in range(n):
                    if fq:
                        fq.popleft()[1]()

            def flush_pt_readers():
                while any(k == "av" for k, _ in fq):
                    fq.popleft()[1]()

            for b in range(B):
                bS = b * S
                if b == 0:
                    # second-half projections + rope b1 ride as fillers
                    for j in range(4, 8):
                        fq.append(("div", lambda j=j: proj_chunk(WQ, QA, j)))
                    for j in range(4, 8):
                        fq.append(("div", lambda j=j: proj_chunk(WK, KA, j)))
                    fq.append(("div", lambda: rope_chunk(2)))
                    fq.append(("div", lambda: rope_chunk(3)))
                    for j in range(4, 8):
                        fq.append(("div", lambda j=j: proj_chunk(WV, VT, j)))

                # V blocked transpose with built-in ones/zero columns:
                # VB0 [p, i, 0:64]=v dims h0, [.,.,64]=1
                # VB1 [p, i, 64:128]=v dims h1, [.,.,32]=1, rest 0
                VB0 = vbp.tile([P, 16, 65], bf, tag="vb0")
                VB1 = vbp.tile([P, 16, P], bf, tag="vb1")
                VBA = vbp.tile([P, 16, 64], bf, tag="vba")
                nc.sync.dma_start_transpose(VBA[:], VT[0:64, bS:bS + S])
                nc.vector.memset(VB0[:, :, 64:65], 1.0)
                nc.vector.tensor_copy(VB0[:, :, 0:64], VBA[:])
                nc.sync.dma_start_transpose(VBA[:], VT[64:128, bS:bS + S])
                nc.vector.memset(VB1[:, :, 0:64], 0.0)
                nc.vector.memset(VB1[:, :, 32:33], 1.0)
                nc.vector.tensor_copy(VB1[:, :, 64:128], VBA[:])

                pts = {}
                pas = {}

                def mk_scores(i, c, pops=1, pts=pts, b=b, bS=bS):
                    qs_i = 512 * (i // 4)
                    q0 = qs_i + 512 * c
                    blk = bS + 128 * i
                    vf = max(0, 128 * i - q0)
                    dc = 128 * i - qs_i
                    pss = []
                    for h in range(HPC):
                        hsl = slice(64 * h, 64 * h + 64)
                        ps = scp.tile([P, 512], f32, tag="sc", name="sc")
                        nc.tensor.matmul(
                            ps[:], lhsT=KA[hsl, blk:blk + 128],
                            rhs=QA[hsl, bS + q0:bS + q0 + 512],
                            start=True, stop=True)
                        pss.append(ps)
                    pop_fill(max(pops, 2 if len(fq) > 10 else 1))
                    for h in range(HPC):
                        pt = pts[(i, h)]
                        o = q0 - qs_i
                        nc.scalar.activation(pt[:, o + vf:o + 512],
                                             pss[h][:, vf:512], EXP,
                                             scale=0.125)
                        if vf > 0:
                            nc.gpsimd.memset(pt[:, o:o + vf], 0.0)
                        if c == 0:
                            nc.gpsimd.tensor_mul(pt[:, dc:dc + 128],
                                                 pt[:, dc:dc + 128], TRI[:])

                def emit_scores(j, pops=1):
                    for i in range(4 * j, 4 * j + 4):
                        for h in range(HPC):
                            pts[(i, h)] = ptp.tile([P, ptw[i]], bf,
                                                   tag=f"pt{i}h{h}",
                                                   name=f"pt{i}h{h}")
                        for c in range(4 - i // 4):
                            mk_scores(i, c, pops)

                def av_chain(j, h, part, pts=pts, pas=pas, VB0=VB0,
                             VB1=VB1):
                    # part 0: i < 4j (ready during scores(j)); part 1: rest
                    ilist = list(range(4 * j)) if part == 0 else \
                        list(range(4 * j, 4 * j + 4))
                    if part == 0 and not ilist:
                        return
                    if part == 0 or j == 0:
                        pa = avp.tile([P, 512], f32, tag=f"pa{h}",
                                      name=f"pa{h}")
                        pas[(j, h)] = pa
                    pa = pas[(j, h)]
                    lhs = (lambda i: VB0[:, i, 0:65]) if h == 0 else \
                          (lambda i: VB1[:, i, :])
                    out = pa[0:65, :] if h == 0 else pa[:, :]
                    for n, i in enumerate(ilist):
                        qs_i = 512 * (i // 4)
                        nc.tensor.matmul(
                            out, lhsT=lhs(i),
                            rhs=pts[(i, h)][:, 512 * j - qs_i:
                                            512 * j - qs_i + 512],
                            start=(part == 0 and n == 0) or (j == 0 and n == 0),
                            stop=(part == 1 and n == len(ilist) - 1),
                            skip_group_check=True)

                def enq_av_heads(j, av=None):
                    av = av or av_chain
                    fq.append(("av", lambda av=av, j=j: av(j, 0, 0)))
                    fq.append(("av", lambda av=av, j=j: av(j, 1, 0)))

                def enq_av(j, b=b, pts=pts, pas=pas):
                    def drains(j=j, pas=pas):
                        pa0, pa1 = pas[(j, 0)], pas[(j, 1)]
                        pj = pjp.tile([P, 512], f32, tag="pasj", name="pj")
                        pas[(j, "sb")] = pj
                        nc.vector.tensor_copy(pj[0:64, :], pa0[0:64, :])
                        nc.vector.tensor_copy(pj[64:128, :], pa1[64:128, :])
                        nc.scalar.copy(SROW[64:65, ts(j, 512)],
                                       pa0[64:65, :])
                        nc.scalar.copy(SROW[32:33, ts(j, 512)],
                                       pa1[32:33, :])

                    fq.append(("av", lambda av=av_chain, j=j: av(j, 0, 1)))
                    fq.append(("av", lambda av=av_chain, j=j: av(j, 1, 1)))
                    fq.append(("av", drains))

                def enq_div(j, b=b, pas=pas):
                    g = j // 2
                    if True:
                        jj = b * 4 + j

                        def recip(j=j, g=g):
                            r = j % 2
                            js = ts(j, 512)
                            nc.sync.dma_start(SUMS4[g][r:r + 1, :],
                                              SROW[32:33, js])
                            nc.sync.dma_start(SUMS4[g][2 + r:3 + r, :],
                                              SROW[64:65, js])
                            nc.vector.reciprocal_approx_fast(RECIP4[g][:],
                                                             SUMS4[g][:])
                        fq.append(("div", recip))

                        def divmul(j=j, g=g, pas=pas):
                            pb = yps.tile([P, 512], f32, tag="y", name="pb")
                            nc.tensor.matmul(pb[:], lhsT=SEL8[:, j % 2, :],
                                             rhs=RECIP4[g][:],
                                             start=True, stop=True)
                            opr = orp.tile([P, 512], bf, tag="opr",
                                           name="opr")
                            pas[(j, "opr")] = opr
                            nc.vector.tensor_mul(opr[:], pas[(j, "sb")][:],
                                                 pb[:])
                        fq.append(("div", divmul))

                        def oproj(j=j, jj=jj, pas=pas):
                            opr = pas[(j, "opr")]
                            for half in range(2):
                                yb = ybp.tile([P, 4, 512], bf, tag="yb",
                                              name="yb")
                                for et in range(4 * half, 4 * half + 4):
                                    py = yps.tile([P, 512], f32, tag="y",
                                                  name="py")
                                    nc.tensor.matmul(py[:],
                                                     lhsT=WO[:, ts(et, P)],
                                                     rhs=opr[:],
                                                     start=True, stop=True)
                                    if jj % 4 == 3 and et % 2 == 0:
                                        nc.scalar.copy(
                                            yb[:, et - 4 * half, :], py[:])
                                    else:
                                        nc.vector.tensor_copy(
                                            yb[:, et - 4 * half, :], py[:])
                                eng = nc.sync if (jj + half) % 2 == 0 \
                                    else nc.scalar
                                eng.dma_start(
                                    yt_d[:, jj, 4 * half:4 * half + 4],
                                    yb[:])
                        fq.append(("div", oproj))

                for j in range(4):
                    if j == 1:
                        flush_pt_readers()
                    enq_av_heads(j)
                    emit_scores(j, pops=1)
                    if b == 0 and j == 0:
                        # drain remaining lead-in, hand PSUM qps -> avp/yps
                        while fq:
                            fq.popleft()[1]()
                        qps.release()
                        avp = tc.alloc_tile_pool(name="avps", bufs=1,
                                                 space="PSUM")
                        yps = tc.alloc_tile_pool(name="yps", bufs=2,
                                                 space="PSUM")
                    enq_av(j)
                    enq_div(j)
            while fq:
                fq.popleft()[1]()
            yps.release()
            avp.release()
        scp.release()
        sws.release()
        swc.release()
        xpB.release()
        xw.release()

    nc.compile()
    return nc


def get_nc():
    global _CACHED_NC
    if _CACHED_NC is None:
        _CACHED_NC = _build_nc()
    return _CACHED_NC


def run_on_hw(in_maps, **kwargs):
    from concourse.bass_utils import run_bass_kernel_spmd
    nc = get_nc()
    return run_bass_kernel_spmd(nc, in_maps, core_ids=list(range(NCORES)),
                                **kwargs)


def kernel(x, token_positions, W_qkv, W_o):
    in_maps = _host_prep(x, token_positions, W_qkv, W_o)
    res = run_on_hw(in_maps)
    acc = np.zeros((D, BS), np.float32)
    for r in res.results:
        yt = np.asarray(r["yt"]).astype(np.float32)      # [p, jj, et, q]
        acc += yt.transpose(2, 0, 1, 3).reshape(D, BS)
    return np.ascontiguousarray(acc.T).reshape(B, S, D).astype(np.float32)
